# revision 17
# baseline (speedup 1.0000x reference)
"""Trainium2 Bass kernel for nn_Attention_Weighted_Context_Generation.

Computes ctx = A @ F where
  A = weights.reshape(9216, 9216)              (row i = output location)
  F = cnn_feature.reshape(256, 9216).T          [9216, 256]
and returns ctx.reshape(9216, 1, 1, 256) float32.

Sharding: rows of A (the HW/location dim) split across 8 NeuronCores,
1152 rows each; F replicated (per the sharding hint). Each core's shard
is packed host-side as one contiguous [9216, 1152+256] float32 array
whose row j holds [A[m0:m1, j] , F[j, :]] — the A-shard transposed, so
the contraction dim lands on SBUF partitions with unit-stride DMA
(TensorE contracts over partitions; A stores the contraction dim
contiguous, so a transpose must happen somewhere — doing it in the
host-side shard packing keeps the device kernel a pure stream).

Device loop: stream 72 k-tiles of [128, 1408] f32 through an SBUF ring
(HWDGE DMAs, ~390 GB/s measured) and accumulate 9 PSUM chains of
[128m, 256c] fp32 across the whole k range (9 matmuls per k-tile,
lhsT/rhs both float32r = full-rate single-pass fp32 mode, ~1.4e-4 rel
err vs the fp32 reference). PSUM is evacuated once at the end (DVE +
ACT split at a bank boundary) and stored with a single DMA.

Measured on trn2 (8 cores): ~153 us/core NEFF exec, ~375-390 GB/s
sustained HBM read per core; memory-roofline bound.
"""

import numpy as np

import concourse.bass as bass
from concourse import mybir
from concourse.bass_utils import run_bass_kernel_spmd

N_CORES = 8
HW = 9216              # number of locations = 96*96
C = 256                # channels
M_PER = HW // N_CORES  # 1152 output rows per core
KT = HW // 128         # 72 contraction tiles
MT = M_PER // 128      # 9 output row-tiles per core
W_COLS = M_PER + C     # 1408 packed columns per k-row
NBUF = 6               # SBUF ring depth for streamed k-tiles
NSEM = 8               # rotation depth for DMA-completion semaphores
DVE_COLS = 6 * C       # PSUM evacuation split (1536 f32 = 3 banks for DVE)

# PE compute dtype. float32r streams 1 output row/cycle at N>=256 (vs 4
# cycles/row for float32) while keeping full fp32 operand bits (TF32-like
# internal precision, measured 1.39e-4 rel err, deterministic). float32
# (exact, ~2x slower end-to-end) and bfloat16 (cast-in-DMA, ~2.6e-3) were
# also validated and can be swapped in here.
COMPUTE_DT = mybir.dt.float32r


def build_bass():
    nc = bass.Bass("TRN2", target_bir_lowering=False, debug=False,
                   num_devices=N_CORES)
    # float32r is bit-identical to float32; declaring the DRAM input as
    # f32r avoids a pointless dtype "cast" in the load DMA.
    atf_dt = (mybir.dt.float32r if COMPUTE_DT == mybir.dt.float32r
              else mybir.dt.float32)
    atf = nc.dram_tensor("atf", [HW, W_COLS], atf_dt,
                         kind="ExternalInput").ap()
    out = nc.dram_tensor("out", [M_PER, C], mybir.dt.float32,
                         kind="ExternalOutput").ap()

    from contextlib import ExitStack
    with (
        ExitStack() as stack,
        nc.sbuf_tensor("kbufs", [128, NBUF * W_COLS], COMPUTE_DT) as kbufs,
        nc.sbuf_tensor("out_sb", [128, MT * C], mybir.dt.float32) as out_sb,
        nc.psum_tensor("acc", [128, MT * C], mybir.dt.float32) as acc,
        nc.semaphore("mm_sem") as mm_sem,
        nc.semaphore("evac_sem") as evac_sem,
        nc.semaphore("out_sem") as out_sem,
        nc.Block() as block,
    ):
        # DMA-completion sems must rotate: a dma_start completes as 16
        # independent per-SDMA-engine increments, and increments of
        # consecutive DMAs interleave across engines. With a single shared
        # sem, "sem >= 16*(jt+1)" does NOT imply DMA jt's data landed
        # (NTFF traces showed the sem leading the last data packet by
        # ~850ns -> stale-tile matmuls, nondeterministic results).
        # Per-engine descriptor FIFO makes a rotation of NSEM sems safe
        # against up to NSEM-1 DMAs of cross-engine skew.
        dma_sems = [stack.enter_context(nc.semaphore(f"dma_sem{i}"))
                    for i in range(NSEM)]

        # fp32->bf16 cast-in-flight would require the SWDGE (gpsimd) DMA
        # path; plain fp32/f32r loads go on the faster HWDGE (sync) path.
        cast_loads = COMPUTE_DT not in (mybir.dt.float32, mybir.dt.float32r)

        def emit_loads(eng):
            for jt in range(KT):
                if jt >= NBUF:
                    # ring slot reused: wait until its matmuls retired
                    eng.wait_ge(mm_sem, jt - NBUF + 1)
                b = jt % NBUF
                eng.dma_start(
                    out=kbufs[:, b * W_COLS:(b + 1) * W_COLS],
                    in_=atf[jt * 128:(jt + 1) * 128, :],
                ).then_inc(dma_sems[jt % NSEM], 16)

        if cast_loads:
            @block.gpsimd
            def _(gpsimd):
                emit_loads(gpsimd)

        @block.sync
        def _(sync):
            if not cast_loads:
                emit_loads(sync)
            sync.wait_ge(evac_sem, 2)
            sync.dma_start(
                out=out.rearrange("(a p) c -> p a c", p=128),
                in_=out_sb[:].rearrange("p (a c) -> p a c", a=MT),
            ).then_inc(out_sem, 16)
            sync.wait_ge(out_sem, 16)

        @block.tensor
        def _(tensor):
            for jt in range(KT):
                tensor.wait_ge(dma_sems[jt % NSEM], 16 * (jt // NSEM + 1))
                b = jt % NBUF
                buf = kbufs[:, b * W_COLS:(b + 1) * W_COLS]
                f_tile = buf[:, M_PER:W_COLS]
                inst = None
                for mi in range(MT):
                    # Two 256-f32 chains share each 512-f32 PSUM bank, and
                    # start=True clears has_written for the WHOLE bank. Only
                    # the bank's first chain (even mi) may clear; the odd
                    # chain's first matmul relies on its bits being clear
                    # already (overwrite-and-set, no bank clear).
                    inst = tensor.matmul(
                        acc[:, mi * C:(mi + 1) * C],
                        buf[:, mi * 128:(mi + 1) * 128],
                        f_tile,
                        start=(jt == 0 and mi % 2 == 0),
                        stop=(jt == KT - 1),
                    )
                inst.then_inc(mm_sem, 1)

        @block.vector
        def _(vector):
            vector.wait_ge(mm_sem, KT)
            vector.tensor_copy(out_sb[:, :DVE_COLS],
                               acc[:, :DVE_COLS]).then_inc(evac_sem, 1)

        @block.scalar
        def _(scalar):
            scalar.wait_ge(mm_sem, KT)
            scalar.copy(out_sb[:, DVE_COLS:],
                        acc[:, DVE_COLS:]).then_inc(evac_sem, 1)

    return nc


def prep_inputs(weights: np.ndarray, cnn_feature: np.ndarray):
    """Pack per-core [9216, 1408] float32 arrays: [A_shard^T | F]."""
    A = np.ascontiguousarray(np.asarray(weights, dtype=np.float32)
                             .reshape(HW, HW))
    F = np.ascontiguousarray(np.asarray(cnn_feature, dtype=np.float32)
                             .reshape(C, HW).T)  # [HW, C]
    in_maps = []
    for i in range(N_CORES):
        at = A[i * M_PER:(i + 1) * M_PER, :].T  # [HW, M_PER] view
        atf = np.concatenate([at, F], axis=1)   # [HW, 1408] contiguous
        in_maps.append({"atf": atf})
    return in_maps


def kernel(weights: np.ndarray, cnn_feature: np.ndarray) -> np.ndarray:
    in_maps = prep_inputs(weights, cnn_feature)
    nc = build_bass()
    res = run_bass_kernel_spmd(nc, in_maps, list(range(N_CORES)))
    ctx = np.concatenate([res.results[i]["out"] for i in range(N_CORES)],
                         axis=0)
    return ctx.reshape(HW, 1, 1, C).astype(np.float32, copy=False)


# revision 21
# speedup vs baseline: 1.0855x; 1.0855x over previous
"""Trainium2 Bass kernel for nn_Attention_Weighted_Context_Generation.

Computes ctx = A @ F where
  A = weights.reshape(9216, 9216)              (row i = output location)
  F = cnn_feature.reshape(256, 9216).T          [9216, 256]
and returns ctx.reshape(9216, 1, 1, 256) float32.

Sharding: rows of A (the HW/location dim) split across 8 NeuronCores,
1152 rows each; F replicated (per the sharding hint). Each core's shard
is packed host-side as one contiguous [9216, 1152+256] float32 array
whose row j holds [A[m0:m1, j] , F[j, :]] — the A-shard transposed, so
the contraction dim lands on SBUF partitions with unit-stride DMA
(TensorE contracts over partitions; A stores the contraction dim
contiguous, so a transpose must happen somewhere — doing it in the
host-side shard packing keeps the device kernel a pure stream).

Device loop: stream 72 k-tiles of [128, 1408] f32 through an SBUF ring
(HWDGE DMAs, ~390 GB/s measured) and accumulate 9 PSUM chains of
[128m, 256c] fp32 across the whole k range (9 matmuls per k-tile,
lhsT/rhs both float32r = full-rate single-pass fp32 mode, ~1.4e-4 rel
err vs the fp32 reference). PSUM is evacuated once at the end (DVE +
ACT split at a bank boundary) and stored with a single DMA.

Measured on trn2 (8 cores): ~153 us/core NEFF exec, ~375-390 GB/s
sustained HBM read per core; memory-roofline bound.
"""

import numpy as np

import concourse.bass as bass
from concourse import mybir
from concourse.bass_utils import run_bass_kernel_spmd

N_CORES = 8
HW = 9216              # number of locations = 96*96
C = 256                # channels
M_PER = HW // N_CORES  # 1152 output rows per core
KT = HW // 128         # 72 contraction tiles
MT = M_PER // 128      # 9 output row-tiles per core
W_COLS = M_PER + C     # 1408 packed columns per k-row
NBUF = 6               # SBUF ring depth for streamed k-tiles
NSEM = 8               # rotation depth for DMA-completion semaphores
DVE_COLS = 6 * C       # PSUM evacuation split (1536 f32 = 3 banks for DVE)

# PE compute dtype. float32r streams 1 output row/cycle at N>=256 (vs 4
# cycles/row for float32) while keeping full fp32 operand bits (TF32-like
# internal precision, measured 1.39e-4 rel err, deterministic). float32
# (exact, ~2x slower end-to-end) and bfloat16 (cast-in-DMA, ~2.6e-3) were
# also validated and can be swapped in here.
COMPUTE_DT = mybir.dt.float32r


def build_bass():
    nc = bass.Bass("TRN2", target_bir_lowering=False, debug=False,
                   num_devices=N_CORES)
    # float32r is bit-identical to float32; declaring the DRAM input as
    # f32r avoids a pointless dtype "cast" in the load DMA.
    atf_dt = (mybir.dt.float32r if COMPUTE_DT == mybir.dt.float32r
              else mybir.dt.float32)
    atf = nc.dram_tensor("atf", [HW, W_COLS], atf_dt,
                         kind="ExternalInput").ap()
    out = nc.dram_tensor("out", [M_PER, C], mybir.dt.float32,
                         kind="ExternalOutput").ap()

    from contextlib import ExitStack
    with (
        ExitStack() as stack,
        nc.sbuf_tensor("kbufs", [128, NBUF * W_COLS], COMPUTE_DT) as kbufs,
        nc.sbuf_tensor("out_sb", [128, MT * C], mybir.dt.float32) as out_sb,
        nc.psum_tensor("acc", [128, MT * C], mybir.dt.float32) as acc,
        nc.semaphore("mm_sem") as mm_sem,
        nc.semaphore("bank_sem") as bank_sem,
        nc.semaphore("dve_done") as dve_done,
        nc.semaphore("act_done") as act_done,
        nc.semaphore("out_sem") as out_sem,
        nc.Block() as block,
    ):
        # DMA-completion sems must rotate: a dma_start completes as 16
        # independent per-SDMA-engine increments, and increments of
        # consecutive DMAs interleave across engines. With a single shared
        # sem, "sem >= 16*(jt+1)" does NOT imply DMA jt's data landed
        # (NTFF traces showed the sem leading the last data packet by
        # ~850ns -> stale-tile matmuls, nondeterministic results).
        # Per-engine descriptor FIFO makes a rotation of NSEM sems safe
        # against up to NSEM-1 DMAs of cross-engine skew.
        dma_sems = [stack.enter_context(nc.semaphore(f"dma_sem{i}"))
                    for i in range(NSEM)]

        # fp32->bf16 cast-in-flight would require the SWDGE (gpsimd) DMA
        # path; plain fp32/f32r loads go on the faster HWDGE (sync) path.
        cast_loads = COMPUTE_DT not in (mybir.dt.float32, mybir.dt.float32r)

        def emit_loads(eng):
            for jt in range(KT):
                if jt >= NBUF:
                    # ring slot reused: wait until its matmuls retired
                    eng.wait_ge(mm_sem, jt - NBUF + 1)
                b = jt % NBUF
                eng.dma_start(
                    out=kbufs[:, b * W_COLS:(b + 1) * W_COLS],
                    in_=atf[jt * 128:(jt + 1) * 128, :],
                ).then_inc(dma_sems[jt % NSEM], 16)

        if cast_loads:
            @block.gpsimd
            def _(gpsimd):
                emit_loads(gpsimd)

        # Split output store: rows 0:768 (chains 0-5, evacuated by DVE) can
        # stream out while ACT still evacuates chains 6-8.
        out_lo = out[:6 * 128, :].rearrange("(a p) c -> p a c", p=128)
        out_hi = out[6 * 128:, :].rearrange("(a p) c -> p a c", p=128)

        @block.sync
        def _(sync):
            if not cast_loads:
                emit_loads(sync)
            sync.wait_ge(dve_done, 1)
            sync.dma_start(
                out=out_lo,
                in_=out_sb[:, :DVE_COLS].rearrange("p (a c) -> p a c", a=6),
            ).then_inc(out_sem, 16)
            sync.wait_ge(act_done, 1)
            sync.dma_start(
                out=out_hi,
                in_=out_sb[:, DVE_COLS:].rearrange("p (a c) -> p a c", a=3),
            ).then_inc(out_sem, 16)
            sync.wait_ge(out_sem, 32)

        @block.tensor
        def _(tensor):
            for jt in range(KT):
                tensor.wait_ge(dma_sems[jt % NSEM], 16 * (jt // NSEM + 1))
                b = jt % NBUF
                buf = kbufs[:, b * W_COLS:(b + 1) * W_COLS]
                f_tile = buf[:, M_PER:W_COLS]
                inst = None
                for mi in range(MT):
                    # Two 256-f32 chains share each 512-f32 PSUM bank, and
                    # start=True clears has_written for the WHOLE bank. Only
                    # the bank's first chain (even mi) may clear; the odd
                    # chain's first matmul relies on its bits being clear
                    # already (overwrite-and-set, no bank clear).
                    inst = tensor.matmul(
                        acc[:, mi * C:(mi + 1) * C],
                        buf[:, mi * 128:(mi + 1) * 128],
                        f_tile,
                        start=(jt == 0 and mi % 2 == 0),
                        stop=(jt == KT - 1),
                    )
                    if jt == KT - 1 and (mi % 2 == 1 or mi == MT - 1):
                        # final group: PSUM bank mi//2 is now final — let the
                        # evac engines start on it while the PE still writes
                        # the higher banks (different banks, collision-safe).
                        inst.then_inc(bank_sem, 1)
                if jt < KT - 1:
                    inst.then_inc(mm_sem, 1)

        @block.vector
        def _(vector):
            # banks 0-2 (chains 0-5), one bank at a time as they finalize
            inst = None
            for b in range(3):
                vector.wait_ge(bank_sem, b + 1)
                inst = vector.tensor_copy(out_sb[:, b * 512:(b + 1) * 512],
                                          acc[:, b * 512:(b + 1) * 512])
            inst.then_inc(dve_done, 1)

        @block.scalar
        def _(scalar):
            # banks 3-4 (chains 6-8)
            scalar.wait_ge(bank_sem, 5)
            scalar.copy(out_sb[:, DVE_COLS:],
                        acc[:, DVE_COLS:]).then_inc(act_done, 1)

    return nc


def prep_inputs(weights: np.ndarray, cnn_feature: np.ndarray):
    """Pack per-core [9216, 1408] float32 arrays: [A_shard^T | F]."""
    A = np.ascontiguousarray(np.asarray(weights, dtype=np.float32)
                             .reshape(HW, HW))
    F = np.ascontiguousarray(np.asarray(cnn_feature, dtype=np.float32)
                             .reshape(C, HW).T)  # [HW, C]
    in_maps = []
    for i in range(N_CORES):
        at = A[i * M_PER:(i + 1) * M_PER, :].T  # [HW, M_PER] view
        atf = np.concatenate([at, F], axis=1)   # [HW, 1408] contiguous
        in_maps.append({"atf": atf})
    return in_maps


def kernel(weights: np.ndarray, cnn_feature: np.ndarray) -> np.ndarray:
    in_maps = prep_inputs(weights, cnn_feature)
    nc = build_bass()
    res = run_bass_kernel_spmd(nc, in_maps, list(range(N_CORES)))
    ctx = np.concatenate([res.results[i]["out"] for i in range(N_CORES)],
                         axis=0)
    return ctx.reshape(HW, 1, 1, C).astype(np.float32, copy=False)
